# revision 1
# baseline (speedup 1.0000x reference)
"""DeepConvAE Trainium2 kernel v2.

Pipeline per sample: 3x conv5x5+relu (f32r matmuls, 1 cyc/row) ->
block-argmax sparsify (DVE + gpsimd partition_all_reduce, zero PE work) ->
deconv1/2/3 as convs over zero-padded inputs with host-flipped weights in
fp8-e4m3 using DoubleRow perf mode (2 taps per pass, 0.5 cyc/row) ->
sigmoid output. Data-parallel over batch: 8 samples/core on 8 cores.

Samples are software-pipelined: encoder of sample b+1 is emitted before
sparsify+decoder of sample b, so the PE never waits on the DVE sparsify
chain. h3/hm/p1 are double-buffered to allow the overlap.
"""

import sys

sys.path.insert(0, "/opt/trn_rl_repo")

import numpy as np

import ml_dtypes
import concourse.bass as bass
import concourse.mybir as mybir
import concourse.tile as tile
from concourse import bass_isa
from concourse import library_config

_e4m3 = np.dtype(ml_dtypes.float8_e4m3)

F32 = mybir.dt.float32
F32R = mybir.dt.float32r
BF16 = mybir.dt.bfloat16
FP8 = mybir.dt.float8e4
DR = mybir.MatmulPerfMode.DoubleRow

N_CORES = 8
BPC = 8
TAPS = [(dy, dx) for dy in range(5) for dx in range(5)]

# fp8 DoubleRow tap pairing: the ifmap pair-step must be EVEN (odd steps
# abort at runtime; step 1/59/177 fail, 2/4/56..144 pass on HW). Vertical
# pairs (step = row pitch) for rows 0-3, two step-2 pairs in row 4, and
# (4,4) left single. Weights are host-reordered to match (PAIR_ORDER).
PAIR_ORDER = (
    [(0, d) for d in range(5) for _ in (0,)] and
    [t for d in range(5) for t in ((0, d), (1, d))]
    + [t for d in range(5) for t in ((2, d), (3, d))]
    + [(4, 0), (4, 2), (4, 1), (4, 3), (4, 4)]
)
PAIR_PERM = [5 * dy + dx for (dy, dx) in PAIR_ORDER]

CONV1 = (64, 64, 60, 60, [8, 8, 8, 8, 7, 7, 7, 7])
CONV2 = (60, 60, 56, 56, [8, 8, 8, 8, 8, 8, 8])
CONV3 = (56, 56, 52, 52, [8, 8, 8, 7, 7, 7, 7])
DECONV1 = (60, 60, 56, 56, [8, 8, 8, 8, 8, 8, 8])
DECONV2 = (64, 64, 60, 60, [8, 8, 8, 8, 7, 7, 7, 7])
DECONV3 = (68, 68, 64, 64, [8, 8, 8, 8, 8, 8, 8, 8])


def _split_waits(nc):
    """Walrus allows at most ONE sync-wait per engine instruction; Tile can
    emit several. Split extras onto same-engine NOPs placed just before."""
    for f in nc.m.functions:
        for blk in f.blocks:
            new = []
            for inst in blk.instructions:
                si = inst.sync_info
                if si is not None and si.on_wait is not None and len(si.on_wait) > 1:
                    waits = list(si.on_wait)
                    for k, w in enumerate(waits[:-1]):
                        nop = mybir.InstNoOp(name=f"{inst.name}_w{k}", ins=[], outs=[])
                        nop.engine = inst.engine
                        nop.sync_info = mybir.SyncInfo(on_wait=[w], on_update=[])
                        new.append(nop)
                    inst.sync_info = mybir.SyncInfo(
                        on_wait=[waits[-1]], on_update=list(si.on_update)
                    )
                new.append(inst)
            blk.instructions = new


def _r0s(chunks):
    r0, out = 0, []
    for r in chunks:
        out.append((r0, r))
        r0 += r
    return out


def _pair_flat(flat, Wi, r0, N, k):
    """[128, 2, N] full-width moving view covering reordered tap pair k:
    dim1 steps by the (even) tap address delta. Out pitch == in pitch, so
    the conv shift is a flat offset; columns past the valid width accumulate
    garbage that the PSUM->SBUF copy skips."""
    dy0, dx0 = PAIR_ORDER[2 * k]
    dy1, dx1 = PAIR_ORDER[2 * k + 1]
    delta = (dy1 - dy0) * Wi + (dx1 - dx0)
    assert delta > 0 and delta % 2 == 0
    o = (r0 + dy0) * Wi + dx0
    v = flat[:, o : o + N].unsqueeze(1).to_broadcast((128, 2, N))
    a = v.ap
    v.ap = a[:1] + [[delta, 2]] + a[2:]
    return v


def build(n_samples=BPC, split_waits=True, repeat=1):
    nc = bass.Bass()
    AF = mybir.ActivationFunctionType

    x_d = nc.dram_tensor("x", [n_samples, 3, 64, 64], F32, kind="ExternalInput")
    w1_d = nc.dram_tensor("w1", [15, 5, 128], F32, kind="ExternalInput")
    w2_d = nc.dram_tensor("w2", [128, 25, 128], F32, kind="ExternalInput")
    w3_d = nc.dram_tensor("w3", [128, 25, 128], F32, kind="ExternalInput")
    w4_d = nc.dram_tensor("w4", [128, 25, 128], FP8, kind="ExternalInput")
    w5_d = nc.dram_tensor("w5", [128, 25, 128], FP8, kind="ExternalInput")
    w6_d = nc.dram_tensor("w6", [128, 128], BF16, kind="ExternalInput")
    sel_d = nc.dram_tensor("sel", [100, 5, 4], BF16, kind="ExternalInput")
    b1_d = nc.dram_tensor("b1", [128, 1], F32, kind="ExternalInput")
    b2_d = nc.dram_tensor("b2", [128, 1], F32, kind="ExternalInput")
    b3_d = nc.dram_tensor("b3", [128, 1], F32, kind="ExternalInput")
    b4_d = nc.dram_tensor("b4", [128, 1], F32, kind="ExternalInput")
    b5_d = nc.dram_tensor("b5", [128, 1], F32, kind="ExternalInput")
    b6_d = nc.dram_tensor("b6", [3, 1], F32, kind="ExternalInput")
    eye_d = nc.dram_tensor("eye", [128, 128], F32, kind="ExternalInput")
    cb_d = nc.dram_tensor("cachebust", [1, 4], F32, kind="ExternalInput")
    if repeat > 1:
        csum_d = nc.dram_tensor("csum", [3, 1], F32, kind="ExternalOutput")
    out_d = nc.dram_tensor("out", [n_samples, 3, 64, 64], F32, kind="ExternalOutput")

    with tile.TileContext(nc) as tc:
        with (
            tc.tile_pool(name="wp", bufs=1) as wp,
            tc.tile_pool(name="act", bufs=1) as ap_,
            tc.tile_pool(name="ps", bufs=8, space=bass.MemorySpace.PSUM) as psp,
        ):
            # ---- resident weights / biases ----
            w1 = wp.tile([15, 5, 128], F32R)
            nc.sync.dma_start(w1[:, :, :], w1_d[:, :, :].bitcast(F32R))
            ws = {}
            for nm, d, dt in (
                ("w2", w2_d, F32R),
                ("w3", w3_d, F32R),
                ("w4", w4_d, FP8),
                ("w5", w5_d, FP8),
            ):
                t = wp.tile([128, 25, 128], dt, tag=nm)
                nc.sync.dma_start(
                    t[:, :, :], d[:, :, :].bitcast(F32R) if dt == F32R else d[:, :, :]
                )
                ws[nm] = t
            w6 = wp.tile([128, 128], BF16)
            nc.sync.dma_start(w6[:, :], w6_d[:, :])
            sel = wp.tile([100, 5, 4], BF16)
            nc.sync.dma_start(sel[:, :, :], sel_d[:, :, :])
            bs = {}
            for nm, d in (("b1", b1_d), ("b2", b2_d), ("b3", b3_d), ("b4", b4_d), ("b5", b5_d)):
                t = wp.tile([128, 1], F32, tag=nm)
                nc.sync.dma_start(t[:, :], d[:, :])
                bs[nm] = t
            b6 = wp.tile([3, 1], F32)
            nc.sync.dma_start(b6[:, :], b6_d[:, :])
            eye = wp.tile([128, 128], F32)
            nc.sync.dma_start(eye[:, :], eye_d[:, :])
            cb = wp.tile([1, 4], F32)
            nc.sync.dma_start(cb[:, :], cb_d[:, :])

            # ---- activation tiles ----
            # conv1 input: 5 dy-shifted copies of x, partition p = 3*dy + ci,
            # 64-pitch full-width (tail-padded for the flat moving slices)
            xd2 = ap_.tile([15, 3844], F32R)
            h1 = ap_.tile([128, 3600], F32R)
            h2 = ap_.tile([128, 3136], F32R)
            h3 = [ap_.tile([128, 2704], F32, name=f"h3_{i}") for i in range(2)]
            hm = [ap_.tile([128, 2704], F32, name=f"hm_{i}") for i in range(2)]
            # padded decoder inputs, flat with a tail pad so full-width
            # moving slices stay in bounds
            p1 = [ap_.tile([128, 3604], FP8, name=f"p1_{i}") for i in range(2)]
            rmax = ap_.tile([128, 1], F32)
            bm1 = ap_.tile([128, 676], F32)
            bm = ap_.tile([128, 169], F32)
            mb = ap_.tile([128, 169], F32)
            mbx = ap_.tile([128, 676], F32)
            rT = ap_.tile([128, 2], F32)
            rrow = ap_.tile([1, 169], F32)
            ones2 = ap_.tile([1, 128], F32)
            nc.vector.memset(ones2[:, :], 1.0)
            p2 = ap_.tile([128, 4100], FP8)
            p3 = ap_.tile([128, 4628], BF16)
            vturn = ap_.tile([100, 4628], BF16)  # per-(tap,co) partial sums
            u1 = ap_.tile([100, 4352], BF16)  # dy-aligned partials, 68-pitch
            o_sb = ap_.tile([3, 4096], F32)

            # zero the padded decoder inputs once; interiors are rewritten
            # per sample and borders never touched again
            for t in (p1[0], p1[1], p2, p3):
                nc.vector.memset(t[:, :], 0.0)
            # xd2's 4-element tail pad is read only by garbage output
            # columns; fill it once with arbitrary finite data (memset
            # cannot emit f32r)
            pad = (
                x_d[0].rearrange("c h w -> c (h w)")[0:3, 0:4]
                .bitcast(F32R).unsqueeze(0).to_broadcast((5, 3, 4))
            )
            nc.sync.dma_start(xd2[:, 3840:3844], pad)
            if repeat > 1:
                csum = ap_.tile([3, 2], F32)
                nc.vector.memset(csum[:, :], 0.0)

            def conv(in_tile, geom, w, bias, dst_fn, func, lo=0, hi=99):
                """f32r conv: 25 tap-matmuls accumulating in PSUM."""
                Hi, Wi, Ho, Wo, chunks = geom
                iv = in_tile[:, :].rearrange("p (h w) -> p h w", w=Wi)
                for r0, R in _r0s(chunks)[lo:hi]:
                    N = R * Wo
                    ps = psp.tile([128, 512], F32, tag="ps", name="ps")
                    for t, (dy, dx) in enumerate(TAPS):
                        nc.tensor.matmul(
                            ps[:, :N],
                            w[:, t, :],
                            iv[:, r0 + dy : r0 + dy + R, dx : dx + Wo],
                            start=(t == 0),
                            stop=(t == 24),
                        )
                    nc.scalar.activation(
                        dst_fn(r0, R),
                        ps[:, :N].rearrange("p (r w) -> p r w", w=Wo),
                        func,
                        bias=bias,
                    )

            def conv_fp8(in_tile, Wi, chunks, w, bias, Wo, dst_fn, func,
                         M=128, Mo=None):
                Mo = M if Mo is None else Mo
                """fp8 DoubleRow conv with full-width moving streams: 12 tap
                pairs at 0.5 cyc/row + tap 24 at 1 cyc/row. Output columns
                Wo..Wi-1 of each row are garbage and skipped on evacuation."""
                flat = in_tile[:, :]
                for r0, R in _r0s(chunks):
                    N = R * Wi
                    ps = psp.tile([128, 512], F32, tag="ps", name="ps")
                    for k in range(12):
                        nc.tensor.matmul(
                            ps[:M, :N],
                            w[:, 2 * k : 2 * k + 2, :],
                            _pair_flat(flat, Wi, r0, N, k),
                            start=(k == 0),
                            stop=False,
                            perf_mode=DR,
                        )
                    o24 = (r0 + 4) * Wi + 4
                    nc.tensor.matmul(
                        ps[:M, :N], w[:, 24, :], flat[:, o24 : o24 + N],
                        start=False, stop=True,
                    )
                    nc.scalar.activation(
                        dst_fn(r0, R),
                        ps[:Mo, :N].rearrange("p (r w) -> p r w", w=Wi)[:, :, 0:Wo],
                        func,
                        bias=bias,
                    )

            def enc(bi, i):
                # conv1: ONE DMA loads 5 dy-shifted copies of x[b] (p=3dy+ci),
                # then 5 accumulation matmuls (one per dx) with K=15 and
                # full-width 64-pitch moving slices
                sv = (
                    x_d[bi].rearrange("c h w -> c (h w)")[0:3, 0:3840]
                    .bitcast(F32R).unsqueeze(0).to_broadcast((5, 3, 3840))
                )
                a = sv.ap
                sv.ap = [[64, 5]] + a[1:]
                nc.sync.dma_start(xd2[:, 0:3840], sv)
                xf = xd2[:, :]
                h1v = h1[:, :].rearrange("p (h w) -> p h w", w=60)
                for r0, R in _r0s([8, 8, 8, 8, 8, 8, 8, 4]):
                    N = R * 64
                    ps = psp.tile([128, 512], F32, tag="ps", name="ps")
                    for dx in range(5):
                        nc.tensor.matmul(
                            ps[:, :N], w1[:, dx, :],
                            xf[:, r0 * 64 + dx : r0 * 64 + dx + N],
                            start=(dx == 0), stop=(dx == 4),
                        )
                    nc.scalar.activation(
                        h1v[:, r0 : r0 + R, :],
                        ps[:, :N].rearrange("p (r w) -> p r w", w=64)[:, :, 0:60],
                        AF.Relu, bias=bs["b1"],
                    )
                h2v = h2[:, :].rearrange("p (h w) -> p h w", w=56)
                conv(h1, CONV2, ws["w2"], bs["b2"],
                     lambda r0, R: h2v[:, r0 : r0 + R, :], AF.Relu)

            def enc3(i, lo, hi):
                h3v = h3[i][:, :].rearrange("p (h w) -> p h w", w=52)
                conv(h2, CONV3, ws["w3"], bs["b3"],
                     lambda r0, R: h3v[:, r0 : r0 + R, :], AF.Relu, lo=lo, hi=hi)

            def spars_dec(bi, i, out_b):
                # ---- sparsify: all on DVE + gpsimd, nothing on PE ----
                X, MAX = mybir.AxisListType.X, mybir.AluOpType.max
                nc.vector.tensor_reduce(rmax[:, :], h3[i][:, :], X, MAX)
                # hm = (h3 == spatial max) * h3
                nc.vector.scalar_tensor_tensor(
                    hm[i][:, :], h3[i][:, :], rmax[:, 0:1], h3[i][:, :],
                    mybir.AluOpType.is_equal, mybir.AluOpType.mult,
                )
                # per-(channel, 4x4 block) max: reduce dy then dx
                nc.vector.tensor_reduce(
                    bm1[:, :],
                    hm[i][:, :].rearrange("p (by dy x) -> p by x dy", by=13, dy=4),
                    X, MAX,
                )
                nc.vector.tensor_reduce(
                    bm[:, :],
                    bm1[:, :].rearrange("p (by bx dx) -> p by bx dx", bx=13, dx=4),
                    X, MAX,
                )
                # cross-channel block max, broadcast to all partitions:
                # PE transpose (channels -> free), DVE reduce, transpose the
                # [128,2] result back to a [2,128] row pair, ones-broadcast
                pst1 = psp.tile([128, 512], F32, tag="ps", name="pst1")
                nc.tensor.transpose(pst1[:, 0:128], bm[:, 0:128], eye[:, :])
                pst2 = psp.tile([128, 512], F32, tag="ps", name="pst2")
                nc.tensor.transpose(pst2[0:41, 0:128], bm[:, 128:169], eye[:, :])
                nc.vector.tensor_reduce(rT[:, 0:1], pst1[:, 0:128], X, MAX)
                nc.vector.tensor_reduce(rT[0:41, 1:2], pst2[0:41, 0:128], X, MAX)
                pst3 = psp.tile([128, 512], F32, tag="ps", name="pst3")
                nc.tensor.transpose(pst3[0:1, 0:128], rT[:, 0:1], eye[:, :])
                pst4 = psp.tile([128, 512], F32, tag="ps", name="pst4")
                nc.tensor.transpose(pst4[0:1, 0:128], rT[:, 1:2], eye[:, :])
                nc.vector.tensor_copy(rrow[0:1, 0:128], pst3[0:1, 0:128])
                nc.vector.tensor_copy(rrow[0:1, 128:169], pst4[0:1, 0:41])
                psb = psp.tile([128, 512], F32, tag="ps", name="psb")
                nc.tensor.matmul(psb[:, 0:169], ones2[0:1, :], rrow[0:1, :],
                                 start=True, stop=True)
                nc.vector.tensor_copy(mb[:, :], psb[:, 0:169])
                # expand blocks along x: mbx[p, by, x] = mb[p, by, x//4]
                nc.vector.tensor_copy(
                    mbx[:, :].rearrange("p (by bx dx) -> p by bx dx", bx=13, dx=4),
                    mb[:, :].rearrange("p (by bx) -> p by bx", bx=13)
                    .unsqueeze(3).to_broadcast((128, 13, 13, 4)),
                )
                # hm <- indicator(hm >= mbx); p1 <- indicator * mbx
                # (where the indicator is 1, hm == mbx == the spike value)
                hmv = hm[i][:, :].rearrange("p (by dy x) -> p by dy x", by=13, dy=4)
                mbv = (
                    mbx[:, :].rearrange("p (by x) -> p by x", x=52)
                    .unsqueeze(2).to_broadcast((128, 13, 4, 52))
                )
                nc.vector.tensor_tensor(hmv, hmv, mbv, mybir.AluOpType.is_ge)
                p1i = (
                    p1[i][:, 244 : 244 + 3120]
                    .rearrange("p (by dy x0) -> p by dy x0", by=13, dy=4)[:, :, :, 0:52]
                )
                nc.vector.tensor_tensor(p1i, hmv, mbv, mybir.AluOpType.mult)

            def dec_main(i):
                # ---- decoder ----
                p2r = p2[:, 0:4096].rearrange("p (h w) -> p h w", w=64)
                conv_fp8(p1[i], 60, DECONV1[4], ws["w4"], bs["b4"], 56,
                         lambda r0, R: p2r[:, 4 + r0 : 4 + r0 + R, 4:60], AF.Relu)
                p3r = p3[:, 0:4624].rearrange("p (h w) -> p h w", w=68)
                conv_fp8(p2, 64, DECONV2[4], ws["w5"], bs["b5"], 60,
                         lambda r0, R: p3r[:, 4 + r0 : 4 + r0 + R, 4:64], AF.Relu)
                # deconv3 one-pass: out[co, y, x] = sum_t v[(t,co), pix+dt]
                # pass 1: one matmul per pixel chunk over ALL (tap, co)
                # columns; evacuate to v; shift-gather per tap via 25 small
                # on-chip DMAs into u; selector matmul sums the taps per co.
                for c0 in range(0, 4624, 512):
                    Nc = min(512, 4624 - c0)
                    ps = psp.tile([128, 512], F32, tag="ps", name="ps")
                    nc.tensor.matmul(ps[:, :Nc], w6[:, :], p3[:, c0 : c0 + Nc],
                                     start=True, stop=True)
                    nc.scalar.activation(vturn[:, c0 : c0 + Nc], ps[0:100, :Nc],
                                         AF.Copy)
                # dy alignment: 5 DMAs copy 20-partition groups with the
                # row shift applied; dx alignment folds into 5 accumulation
                # matmuls against a sparse selector
                vv = vturn[:, 0:4624].rearrange("p (h w) -> p h w", w=68)
                qs = [nc.gpsimd, nc.sync, nc.scalar, nc.gpsimd, nc.sync]
                for dy in range(5):
                    qs[dy].dma_start(
                        u1[20 * dy : 20 * dy + 20, :],
                        vv[20 * dy : 20 * dy + 20, dy : dy + 64, :],
                    )
            def dec_tail(out_b):
                X, MAX = mybir.AxisListType.X, mybir.AluOpType.max
                u1v = u1[:, :].rearrange("p (h w) -> p h w", w=68)
                for ci, c0 in enumerate(range(0, 4096, 512)):
                    r0 = 8 * ci
                    ps = psp.tile([128, 512], F32, tag="ps", name="ps")
                    for dx in range(5):
                        nc.tensor.matmul(ps[0:4, 0:512], sel[:, dx, :],
                                         u1v[:, r0 : r0 + 8, dx : dx + 64],
                                         start=(dx == 0), stop=(dx == 4))
                    nc.scalar.activation(o_sb[0:3, c0 : c0 + 512],
                                         ps[0:3, 0:512], AF.Sigmoid, bias=b6)

                if out_b is None:
                    nc.vector.tensor_reduce(csum[0:3, 1:2], o_sb[:, :], X, MAX)
                    nc.vector.tensor_tensor(csum[0:3, 0:1], csum[0:3, 0:1],
                                            csum[0:3, 1:2], MAX)
                else:
                    nc.sync.dma_start(
                        out_d[out_b].rearrange("co h w -> co (h w)"),
                        o_sb[0:3, :],
                    )

            # ---- software-pipelined sample loop ----
            # emission order interleaves sample b's sparsify/decoder stages
            # between pieces of sample b+1's encoder, so every cross-engine
            # wait (DVE mask chain, deconv3 evac+shift DMAs) is covered by
            # queued PE work
            warm = [(r % n_samples, None) for r in range(repeat - 1)]
            iters = warm + [(b, b) for b in range(n_samples)]
            prev = None
            for it_i, (bi, out_b) in enumerate(iters + [(None, None)]):
                if bi is not None:
                    enc(bi, it_i % 2)
                if prev is not None:
                    spars_dec(prev[0], prev[1], prev[2])
                if bi is not None:
                    enc3(it_i % 2, 0, 4)
                if prev is not None:
                    dec_main(prev[1])
                if bi is not None:
                    enc3(it_i % 2, 4, 7)
                if prev is not None:
                    dec_tail(prev[2])
                prev = (bi, it_i % 2, out_b) if bi is not None else None
            if repeat > 1:
                nc.sync.dma_start(csum_d[:, :], csum[0:3, 0:1])

    if split_waits:
        _split_waits(nc)
    nc.finalize()
    return nc


def _tf32_round(a):
    b = np.ascontiguousarray(a, np.float32).view(np.uint32)
    keep = np.uint32(0xFFFFE000)
    half = np.uint32(0x1000)
    lsb = (b >> np.uint32(13)) & np.uint32(1)
    return ((b + half - np.uint32(1) + lsb) & keep).view(np.float32)


def _make_seldx():
    sel = np.zeros((25, 4, 5, 4), np.float32)
    for t, (dy, dx) in enumerate([(a, b) for a in range(5) for b in range(5)]):
        for co in range(4):
            sel[t, co, dx, co] = 1.0
    return sel.reshape(100, 5, 4)


def _prep_weights(ew0, eb0, ew1, eb1, ew2, eb2, dw0, db0, dw1, db1, dw2, db2):
    f32 = lambda a: np.ascontiguousarray(a, dtype=np.float32)
    f8 = lambda a: np.ascontiguousarray(a, dtype=np.float32).astype(_e4m3)
    bf = lambda a: np.ascontiguousarray(a, dtype=np.float32).astype(
        np.dtype(ml_dtypes.bfloat16))
    return {
        "w1": _tf32_round(np.ascontiguousarray(
            ew0.transpose(2, 1, 3, 0)).reshape(15, 5, 128)),
        "w2": _tf32_round(ew1.transpose(1, 2, 3, 0).reshape(128, 25, 128)),
        "w3": _tf32_round(ew2.transpose(1, 2, 3, 0).reshape(128, 25, 128)),
        "w4": f8(dw0[:, :, ::-1, ::-1].transpose(0, 2, 3, 1)
                 .reshape(128, 25, 128)[:, PAIR_PERM, :]),
        "w5": f8(dw1[:, :, ::-1, ::-1].transpose(0, 2, 3, 1)
                 .reshape(128, 25, 128)[:, PAIR_PERM, :]),
        "w6": bf(np.pad(np.pad(
            dw2[:, :, ::-1, ::-1].transpose(0, 2, 3, 1).reshape(128, 25, 3),
            ((0, 0), (0, 0), (0, 1))).reshape(128, 100), ((0, 0), (0, 28)))),
        "sel": bf(_make_seldx()),
        "b1": f32(eb0.reshape(128, 1)),
        "b2": f32(eb1.reshape(128, 1)),
        "b3": f32(eb2.reshape(128, 1)),
        "b4": f32(db0.reshape(128, 1)),
        "b5": f32(db1.reshape(128, 1)),
        "b6": f32(db2.reshape(3, 1)),
        "eye": np.eye(128, dtype=np.float32),
        "cachebust": np.zeros((1, 4), np.float32),
    }


_NC_CACHE = {}


def _get_nc():
    if "v2" not in _NC_CACHE:
        _NC_CACHE["v2"] = build(BPC)
    return _NC_CACHE["v2"]


def kernel(x, ew0, eb0, ew1, eb1, ew2, eb2, dw0, db0, dw1, db1, dw2, db2,
           trace=False, **_ignored):
    from concourse.bass_utils import run_bass_kernel_spmd

    nc = _get_nc()
    wd = _prep_weights(ew0, eb0, ew1, eb1, ew2, eb2, dw0, db0, dw1, db1, dw2, db2)
    x = np.ascontiguousarray(x, dtype=np.float32)
    in_maps = [dict(wd, x=x[i * BPC : (i + 1) * BPC]) for i in range(N_CORES)]
    r = run_bass_kernel_spmd(nc, in_maps, core_ids=list(range(N_CORES)), trace=trace)
    out = np.concatenate([r.results[i]["out"] for i in range(N_CORES)], axis=0)
    if trace:
        kernel.last_result = r
    return out

